# revision 7
# baseline (speedup 1.0000x reference)
"""Causal GQA self-attention (dense_transformer) on 8 trn2 NeuronCores.

Sharding: core c -> (batch b = c//4, kv-group g = c%4).  Each core computes
the 4 query heads of its kv group against its batch element, plus the
partial output projection for those heads; the host sums the 4 partial
projections per batch (the all-reduce of the tensor-parallel proj).

On-device layout is fully transposed ("feature on partitions"):
  xT [D, S], qhat/khat [head_dim, S], scoresT [k_pos, q_pos], yT [head_dim, S],
  outT [D_out, S].  This makes every matmul contraction land on the partition
  dim with no on-device transposes of activations (RoPE's half-swap is done
  with a permutation matmul, v is transposed head_dim<->seq via the PE).
Softmax is computed without the max subtraction: q/k are RMS-normalized so
|scores| <= gain*sqrt(head_dim) and exp cannot overflow in fp32.
All matmuls run in float32r (~1e-4 rms rounding, full PE rate at N>=256).
"""

import numpy as np

import concourse.bacc as bacc
import concourse.mybir as mybir
import concourse.tile as tile
from concourse.bass_utils import run_bass_kernel_spmd

F32 = mybir.dt.float32
F32R = mybir.dt.float32r
AF = mybir.ActivationFunctionType
ALU = mybir.AluOpType

B = 2
S = 2048
D = 2048
N_HEADS = 16
N_KV = 4
HD = 128
G = N_HEADS // N_KV  # 4 query heads per core
ROPE_BASE = 10000.0
RMS_EPS = 1.1920928955078125e-07
NCH = D // 128       # 16 contraction chunks for the projections
ST = 512             # s-tile width (projection phase)
QT = 512             # q-tile width (attention phase)
KC = 128             # k chunk (scoresT partition block)
GRP = 2              # k chunks per exp group


def _build_program(s_len=S):
    nst = s_len // ST
    nqt = s_len // QT
    nc = bacc.Bacc("TRN2", target_bir_lowering=False, debug=False, num_devices=8)

    xT = nc.dram_tensor("xT", [D, s_len], F32R, kind="ExternalInput")
    wq = nc.dram_tensor("wq", [D, G * HD], F32R, kind="ExternalInput")
    wk = nc.dram_tensor("wk", [D, HD], F32R, kind="ExternalInput")
    wv = nc.dram_tensor("wv", [D, HD], F32R, kind="ExternalInput")
    wp = nc.dram_tensor("wp", [G * HD, D], F32R, kind="ExternalInput")
    cos2 = nc.dram_tensor("cos2", [128, s_len], F32R, kind="ExternalInput")
    sin2 = nc.dram_tensor("sin2", [128, s_len], F32R, kind="ExternalInput")
    gains = nc.dram_tensor("gains", [128, G], F32, kind="ExternalInput")
    swp = nc.dram_tensor("swp", [128, 128], F32R, kind="ExternalInput")
    idn = nc.dram_tensor("idn", [128, 128], F32R, kind="ExternalInput")
    ons = nc.dram_tensor("ons", [128, 128], F32R, kind="ExternalInput")
    msk = nc.dram_tensor("msk", [128, 4 * QT], F32R, kind="ExternalInput")
    out = nc.dram_tensor("out", [D, s_len], F32, kind="ExternalOutput")

    with tile.TileContext(nc) as tc:
        with tc.tile_pool(name="persist", bufs=1) as pp, \
             tc.tile_pool(name="const", bufs=1) as cp:
            qhat = pp.tile([128, G, s_len], F32R)
            khat = pp.tile([128, s_len], F32R)
            vnat = pp.tile([128, s_len // 128, HD], F32R)
            yn = pp.tile([128, G, s_len], F32R)

            swp_sb = cp.tile([128, 128], F32R, tag="swp")
            idn_sb = cp.tile([128, 128], F32R, tag="idn")
            ons_sb = cp.tile([128, 128], F32R, tag="ons")
            gains_sb = cp.tile([128, G], F32, tag="gains")
            msk_sb = cp.tile([128, 4 * QT], F32R, tag="msk")
            epsq_sb = cp.tile([128, 1], F32, tag="epsq")
            epsk_sb = cp.tile([128, 1], F32, tag="epsk")
            nc.gpsimd.memset(epsq_sb[:], RMS_EPS)
            nc.gpsimd.memset(epsk_sb[:], HD * RMS_EPS)
            nc.sync.dma_start(out=swp_sb[:], in_=swp[:])
            nc.sync.dma_start(out=idn_sb[:], in_=idn[:])
            nc.sync.dma_start(out=ons_sb[:], in_=ons[:])
            nc.sync.dma_start(out=gains_sb[:], in_=gains[:])
            nc.sync.dma_start(out=msk_sb[:], in_=msk[:])

            # ---------------- Phase A: projections + RMS norm + RoPE -------
            with tc.tile_pool(name="wA", bufs=1) as wa, \
                 tc.tile_pool(name="xs", bufs=6) as xp, \
                 tc.tile_pool(name="sbA", bufs=2) as sa, \
                 tc.tile_pool(name="psA", bufs=1, space="PSUM") as psa, \
                 tc.tile_pool(name="psAm", bufs=2, space="PSUM") as psm:
                wq_sb = wa.tile([128, NCH, G * HD], F32R, tag="wq")
                wk_sb = wa.tile([128, NCH, HD], F32R, tag="wk")
                wv_sb = wa.tile([128, NCH, HD], F32R, tag="wv")
                cos_sb = wa.tile([128, s_len], F32R, tag="cos")
                sin_sb = wa.tile([128, s_len], F32R, tag="sin")
                nc.sync.dma_start(out=wq_sb[:], in_=wq.rearrange("(c p) m -> p c m", p=128))
                nc.sync.dma_start(out=wk_sb[:], in_=wk.rearrange("(c p) m -> p c m", p=128))
                nc.sync.dma_start(out=wv_sb[:], in_=wv.rearrange("(c p) m -> p c m", p=128))
                nc.sync.dma_start(out=cos_sb[:], in_=cos2[:])
                nc.sync.dma_start(out=sin_sb[:], in_=sin2[:])

                for st in range(nst):
                    s0 = st * ST
                    qp = [psa.tile([128, ST], F32, tag=f"qp{h}", name=f"qp{h}") for h in range(G)]
                    kp = psa.tile([128, ST], F32, tag="kp")
                    vp = psa.tile([128, ST], F32, tag="vp")
                    for c in range(NCH):
                        xs_t = xp.tile([128, ST], F32R)
                        nc.sync.dma_start(out=xs_t[:], in_=xT[c * 128:(c + 1) * 128, s0:s0 + ST])
                        for h in range(G):
                            nc.tensor.matmul(qp[h][:], wq_sb[:, c, h * HD:(h + 1) * HD],
                                             xs_t[:], start=(c == 0), stop=(c == NCH - 1))
                        nc.tensor.matmul(kp[:], wk_sb[:, c, :], xs_t[:],
                                         start=(c == 0), stop=(c == NCH - 1))
                        nc.tensor.matmul(vp[:], wv_sb[:, c, :], xs_t[:],
                                         start=(c == 0), stop=(c == NCH - 1))

                    # q heads and k: drain, RMS stats, RoPE, normalize.
                    for t in range(G + 1):
                        is_q = t < G
                        src = qp[t] if is_q else kp
                        raw = sa.tile([128, ST], F32R, tag="raw")
                        nc.scalar.copy(raw[:], src[:])
                        sq = sa.tile([128, ST], F32R, tag="sq")
                        nc.scalar.activation(sq[:], src[:], AF.Square)
                        smq = psm.tile([128, ST], F32, tag="misc", name="smq")
                        nc.tensor.matmul(smq[:], ons_sb[:], sq[:], start=True, stop=True)
                        den = sa.tile([128, ST], F32, tag="den")
                        if is_q:
                            nc.scalar.activation(den[:], smq[:], AF.Sqrt,
                                                 scale=1.0 / HD, bias=epsq_sb[:, 0:1])
                        else:
                            # fold the 1/sqrt(HD) attention scale into k's norm
                            nc.scalar.activation(den[:], smq[:], AF.Sqrt,
                                                 scale=1.0, bias=epsk_sb[:, 0:1])
                        rcp = sa.tile([128, ST], F32, tag="rcp")
                        nc.vector.reciprocal(rcp[:], den[:])
                        qsw = psm.tile([128, ST], F32, tag="misc", name="qsw")
                        nc.tensor.matmul(qsw[:], swp_sb[:], raw[:], start=True, stop=True)
                        m1 = sa.tile([128, ST], F32, tag="m1")
                        nc.vector.tensor_mul(m1[:], raw[:], cos_sb[:, s0:s0 + ST])
                        m2 = sa.tile([128, ST], F32, tag="m2")
                        nc.vector.tensor_mul(m2[:], qsw[:], sin_sb[:, s0:s0 + ST])
                        rope = sa.tile([128, ST], F32, tag="rope")
                        nc.vector.tensor_add(rope[:], m1[:], m2[:])
                        if is_q:
                            nc.vector.scalar_tensor_tensor(
                                out=qhat[:, t, s0:s0 + ST], in0=rope[:],
                                scalar=gains_sb[:, t:t + 1], in1=rcp[:],
                                op0=ALU.mult, op1=ALU.mult)
                        else:
                            nc.vector.tensor_mul(khat[:, s0:s0 + ST], rope[:], rcp[:])

                    # v: drain + PE-transpose into [s_pos, head_dim] chunks
                    vraw = sa.tile([128, ST], F32R, tag="vraw")
                    nc.scalar.copy(vraw[:], vp[:])
                    for j in range(ST // 128):
                        vtp = psm.tile([128, ST], F32R, tag="misc", name="vtp")
                        nc.tensor.transpose(vtp[:, 0:128], vraw[:, j * 128:(j + 1) * 128], idn_sb[:])
                        nc.scalar.copy(vnat[:, (s0 // 128) + j, :], vtp[:, 0:128])

            # ---------------- Phase B+C: attention + output projection -----
            with tc.tile_pool(name="wB", bufs=1) as wb, \
                 tc.tile_pool(name="sbB", bufs=3) as sb, \
                 tc.tile_pool(name="sbC", bufs=4) as sc_pool, \
                 tc.tile_pool(name="psSC", bufs=2, space="PSUM") as ps_sc, \
                 tc.tile_pool(name="psY", bufs=1, space="PSUM") as ps_y, \
                 tc.tile_pool(name="psSG", bufs=1, space="PSUM") as ps_sg, \
                 tc.tile_pool(name="psC", bufs=2, space="PSUM") as ps_c:
                wp_sb = wb.tile([128, G, D], F32R, tag="wp")
                nc.sync.dma_start(out=wp_sb[:], in_=wp.rearrange("(h p) n -> p h n", p=128))

                for i in range(nqt):
                    q0 = i * QT
                    nch_i = (QT // KC) * (i + 1)  # causal: chunks 0..nch_i-1
                    for h in range(G):
                        yp = ps_y.tile([128, QT], F32, tag="yp")
                        sgp = ps_sg.tile([128, QT], F32, tag="sgp")
                        for g2 in range(nch_i // GRP):
                            scp = ps_sc.tile([128, GRP * QT], F32, tag="scp")
                            for j in range(GRP):
                                c = GRP * g2 + j
                                nc.tensor.matmul(scp[:, j * QT:(j + 1) * QT],
                                                 khat[:, c * KC:(c + 1) * KC],
                                                 qhat[:, h, q0:q0 + QT],
                                                 start=True, stop=True)
                            et = sb.tile([128, GRP * QT], F32R, tag="et")
                            nc.scalar.activation(et[:], scp[:], AF.Exp)
                            jd = GRP * g2 - (QT // KC) * i  # diag offset in mask units
                            if jd + GRP > 0:
                                nc.gpsimd.tensor_mul(
                                    et[:], et[:],
                                    msk_sb[:, jd * QT:(jd + GRP) * QT])
                            for j in range(GRP):
                                c = GRP * g2 + j
                                nc.tensor.matmul(yp[:], vnat[:, c, :],
                                                 et[:, j * QT:(j + 1) * QT],
                                                 start=(c == 0), stop=(c == nch_i - 1))
                                nc.tensor.matmul(sgp[:], ons_sb[:],
                                                 et[:, j * QT:(j + 1) * QT],
                                                 start=(c == 0), stop=(c == nch_i - 1))
                        rs = sb.tile([128, QT], F32, tag="rs")
                        nc.vector.reciprocal(rs[:], sgp[:])
                        nc.vector.tensor_mul(yn[:, h, q0:q0 + QT], yp[:], rs[:])

                    # output projection for this q-tile (all dout chunks)
                    for dc in range(D // 128):
                        op = ps_c.tile([128, QT], F32, tag="op")
                        for h in range(G):
                            nc.tensor.matmul(op[:], wp_sb[:, h, dc * 128:(dc + 1) * 128],
                                             yn[:, h, q0:q0 + QT],
                                             start=(h == 0), stop=(h == G - 1))
                        o_sb = sc_pool.tile([128, QT], F32, tag="osb")
                        if dc % 2 == 0:
                            nc.scalar.copy(o_sb[:], op[:])
                        else:
                            nc.vector.tensor_copy(o_sb[:], op[:])
                        nc.sync.dma_start(out=out[dc * 128:(dc + 1) * 128, q0:q0 + QT],
                                          in_=o_sb[:])
    nc.compile()
    return nc


def _host_tables(s_len=S):
    half = HD // 2
    inv_freq = 1.0 / (ROPE_BASE ** (np.arange(0, HD, 2, dtype=np.float64) / HD))
    t = np.arange(s_len, dtype=np.float64)
    freqs = np.outer(inv_freq, t)  # [64, S]
    c = np.cos(freqs)
    s_ = np.sin(freqs)
    cos2 = np.concatenate([c, c], axis=0).astype(np.float32)          # [128, S]
    sin2 = np.concatenate([s_, -s_], axis=0).astype(np.float32)       # [128, S]
    swp = np.zeros((128, 128), dtype=np.float32)
    swp[np.arange(64), np.arange(64) + 64] = 1.0
    swp[np.arange(64) + 64, np.arange(64)] = 1.0
    idn = np.eye(128, dtype=np.float32)
    ons = np.ones((128, 128), dtype=np.float32)
    # causal masks for the 4 diagonal chunk offsets: keep iff 128*j + p <= f
    p = np.arange(128)[:, None]
    f = np.arange(QT)[None, :]
    msk = np.concatenate(
        [((128 * j + p) <= f).astype(np.float32) for j in range(4)], axis=1)
    return cos2, sin2, swp, idn, ons, msk


_NC_CACHE = {}


def _get_program(s_len=S):
    if s_len not in _NC_CACHE:
        _NC_CACHE[s_len] = _build_program(s_len)
    return _NC_CACHE[s_len]


def make_in_maps(x, Wq, Wk, Wv, Wproj, q_gain, s_len=S):
    x = np.asarray(x, dtype=np.float32)
    Wq = np.asarray(Wq, dtype=np.float32)
    Wk = np.asarray(Wk, dtype=np.float32)
    Wv = np.asarray(Wv, dtype=np.float32)
    Wproj = np.asarray(Wproj, dtype=np.float32)
    q_gain = np.asarray(q_gain, dtype=np.float32)
    cos2, sin2, swp, idn, ons, msk = _host_tables(s_len)
    xT = [np.ascontiguousarray(x[b].T) for b in range(B)]
    in_maps = []
    for core in range(8):
        b, g = core // N_KV, core % N_KV
        in_maps.append({
            "xT": xT[b],
            "wq": np.ascontiguousarray(Wq[g * G * HD:(g + 1) * G * HD, :].T),
            "wk": np.ascontiguousarray(Wk[g * HD:(g + 1) * HD, :].T),
            "wv": np.ascontiguousarray(Wv[g * HD:(g + 1) * HD, :].T),
            "wp": np.ascontiguousarray(Wproj[:, g * G * HD:(g + 1) * G * HD].T),
            "cos2": cos2, "sin2": sin2, "swp": swp, "idn": idn, "ons": ons,
            "msk": msk,
            "gains": np.broadcast_to(q_gain[g * G:(g + 1) * G][None, :],
                                     (128, G)).copy(),
        })
    return in_maps


def unshard(results):
    out = np.empty((B, S, D), dtype=np.float32)
    for b in range(B):
        acc = results[4 * b]["out"].astype(np.float32).copy()
        for g in range(1, N_KV):
            acc += results[4 * b + g]["out"]
        out[b] = acc.T
    return out


def kernel(x, Wq, Wk, Wv, Wproj, q_gain):
    nc = _get_program(S)
    in_maps = make_in_maps(x, Wq, Wk, Wv, Wproj, q_gain, S)
    res = run_bass_kernel_spmd(nc, in_maps, list(range(8)))
    return unshard(res.results)


# revision 24
# speedup vs baseline: 1.6033x; 1.6033x over previous
"""Causal GQA self-attention (dense_transformer) on 8 trn2 NeuronCores.

Sharding: core c -> (batch b = c//4, kv-group g = c%4).  Each core computes
the 4 query heads of its kv group against its batch element, plus the
partial output projection for those heads; the host sums the 4 partial
projections per batch (the all-reduce of the tensor-parallel proj).

On-device layout is fully transposed ("feature on partitions"):
  xT [D, S], qhat/khat [head_dim, S], scoresT [k_pos, q_pos], yT [head_dim, S],
  outT [D_out, S].  This makes every matmul contraction land on the partition
  dim with no on-device transposes of activations (RoPE's half-swap is done
  with a permutation matmul, v is transposed head_dim<->seq via the PE).
Softmax is computed without the max subtraction: q/k are RMS-normalized so
|scores| <= gain*sqrt(head_dim) and exp cannot overflow in fp32.
All matmuls run in float32r (~1e-4 rms rounding, full PE rate at N>=256).
"""

import numpy as np

import concourse.bacc as bacc
import concourse.mybir as mybir
import concourse.tile as tile
from concourse.bass_utils import run_bass_kernel_spmd

F32 = mybir.dt.float32
F32R = mybir.dt.float32r
BF16 = mybir.dt.bfloat16
import os as _os
FAST_DT = BF16 if _os.environ.get("KERNEL_FAST_DT", "bf16") == "bf16" else F32R
AF = mybir.ActivationFunctionType
ALU = mybir.AluOpType

B = 2
S = 2048
D = 2048
N_HEADS = 16
N_KV = 4
HD = 128
G = N_HEADS // N_KV  # 4 query heads per core
ROPE_BASE = 10000.0
RMS_EPS = 1.1920928955078125e-07
NCH = D // 128       # 16 contraction chunks for the projections
ST = 512             # s-tile width (projection phase)
QT = 512             # q-tile width (attention phase)
KC = 128             # k chunk (scoresT partition block)
GRP = 2              # k chunks per exp group


def _build_program(s_len=S):
    nst = s_len // ST
    nqt = s_len // QT
    nc = bacc.Bacc("TRN2", target_bir_lowering=False, debug=False, num_devices=8)

    xT = nc.dram_tensor("xT", [D, s_len], FAST_DT, kind="ExternalInput")
    wq = nc.dram_tensor("wq", [D, G * HD], FAST_DT, kind="ExternalInput")
    wk = nc.dram_tensor("wk", [D, HD], FAST_DT, kind="ExternalInput")
    wv = nc.dram_tensor("wv", [D, HD], FAST_DT, kind="ExternalInput")
    wp = nc.dram_tensor("wp", [G * HD, D], FAST_DT, kind="ExternalInput")
    cos2 = nc.dram_tensor("cos2", [128, s_len], F32R, kind="ExternalInput")
    sin2 = nc.dram_tensor("sin2", [128, s_len], F32R, kind="ExternalInput")
    gains = nc.dram_tensor("gains", [128, G], F32, kind="ExternalInput")
    swp = nc.dram_tensor("swp", [128, 128], FAST_DT, kind="ExternalInput")
    idn = nc.dram_tensor("idn", [128, 128], FAST_DT, kind="ExternalInput")
    ons = nc.dram_tensor("ons", [128, 128], BF16, kind="ExternalInput")
    msk = nc.dram_tensor("msk", [128, 4 * QT], BF16, kind="ExternalInput")
    out = nc.dram_tensor("out", [D, s_len], F32, kind="ExternalOutput")

    with tile.TileContext(nc) as tc:
        with tc.tile_pool(name="persist", bufs=1) as pp, \
             tc.tile_pool(name="const", bufs=1) as cp:
            qhat = pp.tile([128, G, s_len], FAST_DT)
            khat = pp.tile([128, s_len], FAST_DT)
            vnat = pp.tile([128, s_len // 128, HD], BF16)
            yn = pp.tile([128, G, s_len], FAST_DT)

            swp_sb = cp.tile([128, 128], FAST_DT, tag="swp")
            idn_sb = cp.tile([128, 128], FAST_DT, tag="idn")
            ons_sb = cp.tile([128, 128], BF16, tag="ons")
            gains_sb = cp.tile([128, G], F32, tag="gains")
            msk_sb = cp.tile([128, 4 * QT], BF16, tag="msk")
            onsr_sb = cp.tile([128, 128], FAST_DT, tag="onsr")
            nc.gpsimd.memset(onsr_sb[:], 1.0)
            epsq_sb = cp.tile([128, 1], F32, tag="epsq")
            epsk_sb = cp.tile([128, 1], F32, tag="epsk")
            nc.gpsimd.memset(epsq_sb[:], RMS_EPS)
            nc.gpsimd.memset(epsk_sb[:], HD * RMS_EPS)
            warm_sb = cp.tile([128, 1], F32, tag="warm")
            nc.scalar.activation(warm_sb[:], epsq_sb[:], AF.Square)
            nc.scalar.activation(warm_sb[:], epsq_sb[:], AF.Sqrt)
            nc.scalar.activation(warm_sb[:], epsq_sb[:], AF.Exp)
            wp_sb = cp.tile([128, G, D], PR_DT, tag="wp")
            nc.sync.dma_start(out=swp_sb[:], in_=swp[:])
            nc.sync.dma_start(out=idn_sb[:], in_=idn[:])

            # ---------------- Phase A: projections + RMS norm + RoPE -------
            with tc.tile_pool(name="wA", bufs=1) as wa, \
                 tc.tile_pool(name="xs", bufs=6) as xp, \
                 tc.tile_pool(name="sbA", bufs=2) as sa, \
                 tc.tile_pool(name="psA", bufs=1, space="PSUM") as psa, \
                 tc.tile_pool(name="psAm", bufs=2, space="PSUM") as psm:
                wq_sb = wa.tile([128, NCH, G * HD], FAST_DT, tag="wq")
                wk_sb = wa.tile([128, NCH, HD], FAST_DT, tag="wk")
                wv_sb = wa.tile([128, NCH, HD], FAST_DT, tag="wv")
                cos_sb = wa.tile([128, s_len], F32R, tag="cos")
                sin_sb = wa.tile([128, s_len], F32R, tag="sin")
                wqr = wq.rearrange("(c p) m -> p c m", p=128)
                q_ = NCH // 4
                nc.sync.dma_start(out=wq_sb[:, 0:q_, :], in_=wqr[:, 0:q_, :])
                nc.sync.dma_start(out=wk_sb[:, 0:q_, :],
                                  in_=wk.rearrange("(c p) m -> p c m", p=128)[:, 0:q_, :])
                nc.sync.dma_start(out=wv_sb[:, 0:q_, :],
                                  in_=wv.rearrange("(c p) m -> p c m", p=128)[:, 0:q_, :])

                for st in range(nst):
                    s0 = st * ST
                    qp = [psa.tile([128, ST], F32, tag=f"qp{h}", name=f"qp{h}") for h in range(G)]
                    kp = psa.tile([128, ST], F32, tag="kp")
                    vp = psa.tile([128, ST], F32, tag="vp")
                    for c in range(NCH):
                        xs_t = xp.tile([128, ST], FAST_DT)
                        nc.sync.dma_start(out=xs_t[:], in_=xT[c * 128:(c + 1) * 128, s0:s0 + ST])
                        for h in range(G):
                            nc.tensor.matmul(qp[h][:], wq_sb[:, c, h * HD:(h + 1) * HD],
                                             xs_t[:], start=(c == 0), stop=(c == NCH - 1))
                        nc.tensor.matmul(kp[:], wk_sb[:, c, :], xs_t[:],
                                         start=(c == 0), stop=(c == NCH - 1))
                        nc.tensor.matmul(vp[:], wv_sb[:, c, :], xs_t[:],
                                         start=(c == 0), stop=(c == NCH - 1))

                    # q heads and k: drain, RMS stats, RoPE, normalize.
                    for t in range(G + 1):
                        is_q = t < G
                        src = qp[t] if is_q else kp
                        raw = sa.tile([128, ST], FAST_DT, tag="raw")
                        nc.scalar.copy(raw[:], src[:])
                        sq = sa.tile([128, ST], FAST_DT, tag="sq")
                        nc.gpsimd.tensor_mul(sq[:], raw[:], raw[:])
                        smq = psm.tile([128, ST], F32, tag="misc", name="smq")
                        nc.tensor.matmul(smq[:], onsr_sb[:], sq[:], start=True, stop=True)
                        den = sa.tile([128, ST], F32, tag="den")
                        if is_q:
                            nc.scalar.activation(den[:], smq[:], AF.Sqrt,
                                                 scale=1.0 / HD, bias=epsq_sb[:, 0:1])
                        else:
                            # fold the 1/sqrt(HD) attention scale into k's norm
                            nc.scalar.activation(den[:], smq[:], AF.Sqrt,
                                                 scale=1.0, bias=epsk_sb[:, 0:1])
                        rcp = sa.tile([128, ST], F32, tag="rcp")
                        nc.vector.reciprocal_approx_fast(rcp[:], den[:])
                        qsw = psm.tile([128, ST], F32, tag="misc", name="qsw")
                        nc.tensor.matmul(qsw[:], swp_sb[:], raw[:], start=True, stop=True)
                        m1 = sa.tile([128, ST], F32, tag="m1")
                        nc.vector.tensor_mul(m1[:], raw[:], cos_sb[:, s0:s0 + ST])
                        m2 = sa.tile([128, ST], F32, tag="m2")
                        nc.vector.tensor_mul(m2[:], qsw[:], sin_sb[:, s0:s0 + ST])
                        rope = sa.tile([128, ST], F32, tag="rope")
                        nc.gpsimd.tensor_add(rope[:], m1[:], m2[:])
                        if is_q:
                            nc.vector.scalar_tensor_tensor(
                                out=qhat[:, t, s0:s0 + ST], in0=rope[:],
                                scalar=gains_sb[:, t:t + 1], in1=rcp[:],
                                op0=ALU.mult, op1=ALU.mult)
                        else:
                            nc.gpsimd.tensor_mul(khat[:, s0:s0 + ST], rope[:], rcp[:])

                    # v: drain + PE-transpose into [s_pos, head_dim] chunks
                    vraw = sa.tile([128, ST], FAST_DT, tag="vraw")
                    nc.scalar.copy(vraw[:], vp[:])
                    for j in range(ST // 128):
                        vtp = psm.tile([128, ST], FAST_DT, tag="misc", name="vtp")
                        nc.tensor.transpose(vtp[:, 0:128], vraw[:, j * 128:(j + 1) * 128], idn_sb[:])
                        nc.scalar.copy(vnat[:, (s0 // 128) + j, :], vtp[:, 0:128])

            # ---------------- Phase B+C: attention + output projection -----
            with tc.tile_pool(name="sbB", bufs=3) as sb, \
                 tc.tile_pool(name="sbC", bufs=4) as sc_pool, \
                 tc.tile_pool(name="psY", bufs=1, space="PSUM") as ps_y, \
                 tc.tile_pool(name="psSG", bufs=1, space="PSUM") as ps_sg, \
                 tc.tile_pool(name="psSC", bufs=3, space="PSUM") as ps_sc:
                wp_sb = wb.tile([128, G, D], FAST_DT, tag="wp")

                for i in list(range(1, nqt)) + [0]:
                    q0 = i * QT
                    nch_i = (QT // KC) * (i + 1)  # causal: chunks 0..nch_i-1
                    for h in range(G):
                        yp_t = ps_y.tile([128, QT], F32, tag="yp")
                        sgp_t = ps_sg.tile([128, QT], F32, tag="sgp")
                        yp = yp_t[:]
                        sgp = sgp_t[:]
                        for g2 in range(nch_i // GRP):
                            scp = ps_sc.tile([128, GRP * QT], F32, tag="scp")
                            for j in range(GRP):
                                c = GRP * g2 + j
                                nc.tensor.matmul(scp[:, j * QT:(j + 1) * QT],
                                                 khat[:, c * KC:(c + 1) * KC],
                                                 qhat[:, h, q0:q0 + QT],
                                                 start=True, stop=True)
                            et = sb.tile([128, GRP * QT], BF16, tag="et")
                            nc.scalar.activation(et[:], scp[:], AF.Exp)
                            jd = GRP * g2 - (QT // KC) * i  # diag offset in mask units
                            if jd + GRP > 0:
                                nc.gpsimd.tensor_mul(
                                    et[:], et[:],
                                    msk_sb[:, jd * QT:(jd + GRP) * QT])
                            for j in range(GRP):
                                c = GRP * g2 + j
                                nc.tensor.matmul(yp, vnat[:, c, :],
                                                 et[:, j * QT:(j + 1) * QT],
                                                 start=(c == 0), stop=(c == nch_i - 1))
                                nc.tensor.matmul(sgp, ons_sb[:],
                                                 et[:, j * QT:(j + 1) * QT],
                                                 start=(c == 0), stop=(c == nch_i - 1))
                        rs = sb.tile([128, QT], F32, tag="rs")
                        nc.vector.reciprocal_approx_fast(rs[:], sgp)
                        nc.vector.tensor_mul(yn[:, h, q0:q0 + QT], yp, rs[:])

                    # output projection for this q-tile (all dout chunks)
                    o_acc = sc_pool.tile([128, D // 128, QT], F32, tag="osb", bufs=2)
                    outr = out.rearrange("(dc p) q -> p dc q", p=128)
                    for dc in range(D // 128):
                        op_t = ps_sc.tile([128, GRP * QT], F32, tag="scp", name="op_t")
                        op = op_t[:, 0:QT]
                        for h in range(G):
                            nc.tensor.matmul(op, wp_sb[:, h, dc * 128:(dc + 1) * 128],
                                             yn[:, h, q0:q0 + QT],
                                             start=(h == 0), stop=(h == G - 1))
                        nc.vector.tensor_copy(o_acc[:, dc, :], op)
                        if dc == 7:
                            nc.sync.dma_start(out=outr[:, 0:8, q0:q0 + QT],
                                              in_=o_acc[:, 0:8, :])
                    nc.sync.dma_start(out=outr[:, 8:16, q0:q0 + QT],
                                      in_=o_acc[:, 8:16, :])
    nc.compile()
    return nc


def _host_tables(s_len=S):
    half = HD // 2
    inv_freq = 1.0 / (ROPE_BASE ** (np.arange(0, HD, 2, dtype=np.float64) / HD))
    t = np.arange(s_len, dtype=np.float64)
    freqs = np.outer(inv_freq, t)  # [64, S]
    c = np.cos(freqs)
    s_ = np.sin(freqs)
    cos2 = np.concatenate([c, c], axis=0).astype(np.float32)          # [128, S]
    sin2 = np.concatenate([s_, -s_], axis=0).astype(np.float32)       # [128, S]
    swp = np.zeros((128, 128), dtype=np.float32)
    swp[np.arange(64), np.arange(64) + 64] = 1.0
    swp[np.arange(64) + 64, np.arange(64)] = 1.0
    idn = np.eye(128, dtype=np.float32)
    ons = np.ones((128, 128), dtype=np.float32)
    # causal masks for the 4 diagonal chunk offsets: keep iff 128*j + p <= f
    p = np.arange(128)[:, None]
    f = np.arange(QT)[None, :]
    msk = np.concatenate(
        [((128 * j + p) <= f).astype(np.float32) for j in range(4)], axis=1)
    import ml_dtypes
    ons = ons.astype(ml_dtypes.bfloat16)
    msk = msk.astype(ml_dtypes.bfloat16)
    return cos2, sin2, swp, idn, ons, msk


_NC_CACHE = {}


def _get_program(s_len=S):
    if s_len not in _NC_CACHE:
        _NC_CACHE[s_len] = _build_program(s_len)
    return _NC_CACHE[s_len]


def _fast_np(a):
    import ml_dtypes
    if FAST_DT == BF16:
        return np.ascontiguousarray(a).astype(ml_dtypes.bfloat16)
    return np.ascontiguousarray(a)


def make_in_maps(x, Wq, Wk, Wv, Wproj, q_gain, s_len=S):
    x = np.asarray(x, dtype=np.float32)
    Wq = np.asarray(Wq, dtype=np.float32)
    Wk = np.asarray(Wk, dtype=np.float32)
    Wv = np.asarray(Wv, dtype=np.float32)
    Wproj = np.asarray(Wproj, dtype=np.float32)
    q_gain = np.asarray(q_gain, dtype=np.float32)
    cos2, sin2, swp, idn, ons, msk = _host_tables(s_len)
    xT = [np.ascontiguousarray(x[b].T) for b in range(B)]
    in_maps = []
    for core in range(8):
        b, g = core // N_KV, core % N_KV
        in_maps.append({
            "xT": _fast_np(xT[b]),
            "wq": _fast_np(Wq[g * G * HD:(g + 1) * G * HD, :].T),
            "wk": _fast_np(Wk[g * HD:(g + 1) * HD, :].T),
            "wv": _fast_np(Wv[g * HD:(g + 1) * HD, :].T),
            "wp": _fast_np(Wproj[:, g * G * HD:(g + 1) * G * HD].T),
            "cos2": cos2, "sin2": sin2, "swp": _fast_np(swp), "idn": _fast_np(idn),
            "ons": ons,
            "msk": msk,
            "gains": np.broadcast_to(q_gain[g * G:(g + 1) * G][None, :],
                                     (128, G)).copy(),
        })
    return in_maps


def unshard(results):
    out = np.empty((B, S, D), dtype=np.float32)
    for b in range(B):
        acc = results[4 * b]["out"].astype(np.float32).copy()
        for g in range(1, N_KV):
            acc += results[4 * b + g]["out"]
        out[b] = acc.T
    return out


def kernel(x, Wq, Wk, Wv, Wproj, q_gain):
    nc = _get_program(S)
    in_maps = make_in_maps(x, Wq, Wk, Wv, Wproj, q_gain, S)
    res = run_bass_kernel_spmd(nc, in_maps, list(range(8)))
    return unshard(res.results)
